# revision 14
# baseline (speedup 1.0000x reference)
"""Trainium2 Bass kernel for a 2-layer LSTM (MnistModel frames).

Model: xb [2048, 8192] -> frames [T=64, B, 128] -> LSTM(128->512) ->
LSTM(512->512) -> last hidden -> Linear(512->10).

Sharding: data-parallel over batch (2048 -> 256 per core, 8 cores),
weights replicated.  Everything on-chip lives transposed ([feature,
batch]); weights/x are PE-transposed at startup and cast to bf16 (full
PE rate, fast-weight-load active).  All 64 transposed x timesteps stay
resident in SBUF as bf16 (32KB/partition).  Startup is interleaved with
the recurrence: weight staging DMAs are issued up front (by k-chunk),
transposes and x prep stream between the first timesteps, and the first
timestep of each layer skips the h-matmuls (h0 == 0).  Layer 1 runs LAG
timesteps behind layer 0 and the two layers' PSUM group pipelines are
emitted interleaved.
"""

import os
import sys

import numpy as np

for _p in ("/opt/trn_rl_repo", "/root/.axon_site/_ro/trn_rl_repo"):
    if os.path.isdir(_p) and _p not in sys.path:
        sys.path.insert(0, _p)

import concourse.bass as bass  # noqa: E402
import concourse.mybir as mybir  # noqa: E402
import concourse.tile as tile  # noqa: E402
from concourse import bacc  # noqa: E402
from concourse.bass_utils import run_bass_kernel_spmd  # noqa: E402
from concourse.masks import make_identity  # noqa: E402

F32 = mybir.dt.float32
BF16 = mybir.dt.bfloat16
AF = mybir.ActivationFunctionType

B, L, IN, H, OUT = 2048, 8192, 128, 512, 10
T = L // IN  # 64 timesteps
NCORES = 8
BL = B // NCORES  # 256 batch rows per core
G4 = 4 * H  # 2048 gate rows
NKC = H // 128  # 4 hidden k-chunks
NMC = G4 // 128  # 16 gate m-chunks
NB = BL  # matmul moving free dim
CHUNK_T = 8  # x timesteps per staging tile

_CACHE = {}


def _build(opts=None):
    _defaults = dict(
        wk_bufs=2,
        st_bufs=2,
        k_outer=True,
        lag=2,
    )
    _defaults.update(opts or {})
    opts = _defaults
    LAG = opts["lag"]
    assert LAG >= 1

    nc = bacc.Bacc()
    xb = nc.declare_dram_parameter("xb", [BL, L], F32, isOutput=False)
    W_ih0 = nc.declare_dram_parameter("W_ih0", [G4, IN], F32, isOutput=False)
    W_hh0 = nc.declare_dram_parameter("W_hh0", [G4, H], F32, isOutput=False)
    b0 = nc.declare_dram_parameter("b0", [G4], F32, isOutput=False)
    W_ih1 = nc.declare_dram_parameter("W_ih1", [G4, H], F32, isOutput=False)
    W_hh1 = nc.declare_dram_parameter("W_hh1", [G4, H], F32, isOutput=False)
    b1 = nc.declare_dram_parameter("b1", [G4], F32, isOutput=False)
    W_out = nc.declare_dram_parameter("W_out", [OUT, H], F32, isOutput=False)
    b_out = nc.declare_dram_parameter("b_out", [OUT], F32, isOutput=False)
    out = nc.declare_dram_parameter("out", [BL, OUT], F32, isOutput=True)

    with tile.TileContext(nc) as tc:
        with (
            tc.tile_pool(name="const", bufs=1) as const,
            tc.tile_pool(name="wstg", bufs=1) as wstg,
            tc.tile_pool(name="xstg", bufs=1) as xstg,
            tc.tile_pool(name="ps0", bufs=2, space="PSUM") as ps0,
            tc.tile_pool(name="ps1", bufs=2, space="PSUM") as ps1,
            tc.tile_pool(name="state", bufs=opts["st_bufs"]) as stp,
            tc.tile_pool(name="work", bufs=opts["wk_bufs"]) as wkp,
        ):
            ident = const.tile([128, 128], F32, tag="ident")
            make_identity(nc, ident)

            b0t = const.tile([128, NMC], F32, tag="b0t")
            nc.sync.dma_start(out=b0t, in_=b0[:].rearrange("(m p) -> p m", p=128))
            b1t = const.tile([128, NMC], F32, tag="b1t")
            nc.sync.dma_start(out=b1t, in_=b1[:].rearrange("(m p) -> p m", p=128))
            bot = const.tile([OUT, 1], F32, tag="bot")
            nc.sync.dma_start(out=bot, in_=b_out[:].rearrange("(p o) -> p o", o=1))

            # all transposed x timesteps live in SBUF as bf16
            xts = [
                const.tile([128, NB], BF16, tag=f"xts{t}", name=f"xts{t}")
                for t in range(T)
            ]

            # ---- weight staging: issue all DMAs up front (by k-chunk);
            # transposes stream later between recurrence timesteps.
            def load_start(wd, kdim, name):
                """Returns (bf16 lhsT tiles per k-chunk, staged fp32 tiles)."""
                nkc = kdim // 128
                wts = [
                    const.tile(
                        [128, G4], BF16, tag=f"{name}_{kc}", name=f"{name}_{kc}"
                    )
                    for kc in range(nkc)
                ]
                wv = wd[:, :].rearrange("(g p) k -> p g k", p=128)
                sts = []
                for kc in range(nkc):
                    st = wstg.tile(
                        [128, NMC, 128],
                        F32,
                        tag="wst",
                        name=f"st_{name}{kc}",
                        bufs=4,
                    )
                    nc.scalar.dma_start(
                        out=st, in_=wv[:, :, kc * 128 : (kc + 1) * 128]
                    )
                    sts.append(st)
                return wts, sts

            def wT_gen(wts, sts):
                """PE-transpose staged [128, 16, 128] chunks into bf16 lhsT
                tiles; yields between 512-col groups."""
                for kc, st in enumerate(sts):
                    for mg in range(NMC // 4):
                        pt = ps1.tile(
                            [128, 4 * NB], F32, tag="g", name="wpt"
                        )[:, 0:512]
                        for j in range(4):
                            mc = mg * 4 + j
                            nc.tensor.transpose(
                                pt[:, j * 128 : (j + 1) * 128],
                                st[:, mc, :],
                                ident,
                            )
                        nc.vector.tensor_copy(
                            wts[kc][:, mg * 512 : (mg + 1) * 512], pt
                        )
                        yield

            def x_group_gen(g):
                """Stage CHUNK_T timesteps of x, transpose to bf16 SBUF."""
                tch = g * CHUNK_T
                tend = min(tch + CHUNK_T, T)
                n = (tend - tch) * IN
                xs0 = xstg.tile([128, n], F32, tag="xs0", name=f"xs0_{g}", bufs=2)
                xs1 = xstg.tile([128, n], F32, tag="xs1", name=f"xs1_{g}", bufs=2)
                nc.sync.dma_start(out=xs0, in_=xb[0:128, tch * IN : tend * IN])
                nc.sync.dma_start(out=xs1, in_=xb[128:256, tch * IN : tend * IN])
                for t in range(tch, tend):
                    off = (t - tch) * IN
                    pt = ps0.tile([128, 4 * NB], F32, tag="g", name="xpt")[
                        :, 0:NB
                    ]
                    nc.tensor.transpose(pt[:, 0:128], xs0[:, off : off + IN], ident)
                    nc.tensor.transpose(
                        pt[:, 128:256], xs1[:, off : off + IN], ident
                    )
                    nc.vector.tensor_copy(xts[t], pt)
                    if t % 2 == 1:
                        yield

            wts_ih0, sts_ih0 = load_start(W_ih0, IN, "wih0")
            wts_hh0, sts_hh0 = load_start(W_hh0, H, "whh0")
            wts_ih1, sts_ih1 = load_start(W_ih1, H, "wih1")
            wts_hh1, sts_hh1 = load_start(W_hh1, H, "whh1")
            WT_ih0 = wts_ih0[0]

            def drive(gens):
                alive = list(gens)
                while alive:
                    for g in list(alive):
                        try:
                            next(g)
                        except StopIteration:
                            alive.remove(g)

            h0, c0, h1, c1 = [None] * NKC, [None] * NKC, [None] * NKC, [None] * NKC

            def lstm_step_gen(lname, pspool, pairs, c_prev, bt, res):
                """One LSTM layer timestep, transposed layout; yields after
                each of the NKC gate groups so two layers interleave.

                PSUM group tile holds (i|f|o|g) for one 128-slice of the
                hidden dim; all `pairs` (wT, rhs) accumulate into it, k-outer
                so late-arriving rhs chunks are needed late.  c_prev is None
                on the first timestep (h==0): f/v are skipped."""
                first = c_prev is None
                h_new, c_new = [], []
                n = len(pairs)
                for p in range(NKC):
                    ps = pspool.tile([128, 4 * NB], F32, tag="g", name=f"ps{lname}")
                    if opts["k_outer"]:
                        # k-outer across the two PSUM banks of the group
                        # tile: regions (i, o) accumulate k-interleaved,
                        # then (f, g).  Within a bank each region's
                        # accumulation stays contiguous.
                        for sub in range(2):
                            for idx, (wt, rhs) in enumerate(pairs):
                                for pos in (sub, sub + 2):
                                    gate = (0, 1, 3, 2)[pos]
                                    if first and gate == 1:
                                        continue  # f unused when c==0
                                    mc = gate * NKC + p
                                    nc.tensor.matmul(
                                        ps[:, pos * NB : (pos + 1) * NB],
                                        wt[:, mc * 128 : (mc + 1) * 128],
                                        rhs,
                                        start=(idx == 0),
                                        stop=(idx == n - 1),
                                        skip_group_check=True,
                                    )
                    else:
                        for pos, gate in enumerate((0, 1, 3, 2)):
                            if first and gate == 1:
                                continue
                            mc = gate * NKC + p
                            for idx, (wt, rhs) in enumerate(pairs):
                                nc.tensor.matmul(
                                    ps[:, pos * NB : (pos + 1) * NB],
                                    wt[:, mc * 128 : (mc + 1) * 128],
                                    rhs,
                                    start=(idx == 0),
                                    stop=(idx == n - 1),
                                )
                    # ACT order: i, f, g, o, then tanh(c) — so sigmoid(o)
                    # overlaps the DVE u/v/c chain and tanh(c) is reached
                    # as early as possible (h critical path).
                    sg = wkp.tile([128, 3 * NB], BF16, tag=f"sg{lname}")
                    nc.scalar.activation(
                        sg[:, 0:NB], ps[:, 0:NB], AF.Sigmoid,
                        bias=bt[:, p : p + 1],
                    )
                    if not first:
                        mcf = NKC + p
                        nc.scalar.activation(
                            sg[:, NB : 2 * NB], ps[:, NB : 2 * NB], AF.Sigmoid,
                            bias=bt[:, mcf : mcf + 1],
                        )
                    tg = wkp.tile([128, NB], BF16, tag=f"tg{lname}")
                    mcg = 2 * NKC + p
                    nc.scalar.activation(
                        tg, ps[:, 3 * NB : 4 * NB], AF.Tanh,
                        bias=bt[:, mcg : mcg + 1],
                    )
                    cn = stp.tile([128, NB], BF16, tag=f"c{lname}_{p}", bufs=2)
                    if first:
                        nc.vector.tensor_mul(cn, sg[:, 0:NB], tg)
                    else:
                        u = wkp.tile([128, NB], BF16, tag=f"u{lname}")
                        nc.vector.tensor_mul(u, sg[:, 0:NB], tg)
                        v = wkp.tile([128, NB], BF16, tag=f"v{lname}")
                        nc.vector.tensor_mul(v, sg[:, NB : 2 * NB], c_prev[p])
                        nc.vector.tensor_add(cn, u, v)
                    mco = 3 * NKC + p
                    nc.scalar.activation(
                        sg[:, 2 * NB : 3 * NB], ps[:, 2 * NB : 3 * NB],
                        AF.Sigmoid, bias=bt[:, mco : mco + 1],
                    )
                    th = wkp.tile([128, NB], BF16, tag=f"th{lname}")
                    nc.scalar.activation(th, cn, AF.Tanh)
                    hn = stp.tile(
                        [128, NB], BF16, tag=f"h{lname}_{p}",
                        bufs=(LAG + 2) if lname == "0" else 2,
                    )
                    nc.vector.tensor_mul(hn, sg[:, 2 * NB : 3 * NB], th)
                    h_new.append(hn)
                    c_new.append(cn)
                    yield
                res[lname] = (h_new, c_new)

            hs0 = {}  # t -> h0 chunks (consumed by layer 1 at t)

            def emit_l0(t):
                nonlocal h0, c0
                if t == 0:
                    pairs = [(WT_ih0, xts[0])]
                    cp = None
                else:
                    pairs = [(WT_ih0, xts[t])] + [
                        (wts_hh0[kc], h0[kc]) for kc in range(NKC)
                    ]
                    cp = c0
                res = {}
                yield from lstm_step_gen("0", ps0, pairs, cp, b0t, res)
                h0, c0 = res["0"]
                hs0[t] = h0

            def emit_l1(t):
                nonlocal h1, c1
                h0t = hs0.pop(t)
                if t == 0:
                    pairs = [(wts_ih1[kc], h0t[kc]) for kc in range(NKC)]
                    cp = None
                else:
                    pairs = [(wts_ih1[kc], h0t[kc]) for kc in range(NKC)] + [
                        (wts_hh1[kc], h1[kc]) for kc in range(NKC)
                    ]
                    cp = c1
                res = {}
                yield from lstm_step_gen("1", ps1, pairs, cp, b1t, res)
                h1, c1 = res["1"]

            # ---- prologue: x group 0 + W_ih0 transposed before t0; the
            # rest of the weight transposes and x groups stream between
            # the first timesteps (staging DMAs are already in flight).
            drive([x_group_gen(0)])
            drive([wT_gen(wts_ih0, sts_ih0)])
            drive([x_group_gen(1)])
            drive([emit_l0(0)])
            drive([wT_gen(wts_hh0, sts_hh0)])
            sched = {
                1: [x_group_gen(2)],
                2: [x_group_gen(3), wT_gen(wts_ih1, sts_ih1)],
                3: [x_group_gen(4)],
                4: [x_group_gen(5), wT_gen(wts_hh1, sts_hh1)],
                5: [x_group_gen(6)],
                6: [x_group_gen(7)],
            }
            for t in range(1, LAG):
                drive([emit_l0(t)] + sched.pop(t, []))
            for t in range(LAG, T):
                drive([emit_l0(t), emit_l1(t - LAG)] + sched.pop(t, []))
            for t in range(T - LAG, T):
                drive([emit_l1(t)])

            # head: out.T [10, 256] = W_out @ h1T + b_out
            WT_out = const.tile([128, NKC * OUT], BF16, tag="wout")
            stw = wstg.tile([OUT, H], F32, tag="stw", name="st_wo")
            nc.scalar.dma_start(out=stw, in_=W_out[:, :])
            for kc in range(NKC):
                pt = ps0.tile([128, 4 * NB], F32, tag="g", name="ps0w")[
                    :, 0:OUT
                ]
                nc.tensor.transpose(
                    pt, stw[:, kc * 128 : (kc + 1) * 128], ident[:OUT, :OUT]
                )
                nc.vector.tensor_copy(WT_out[:, kc * OUT : (kc + 1) * OUT], pt)
            psf = ps0.tile([128, 4 * NB], F32, tag="g", name="psf")
            for kc in range(NKC):
                nc.tensor.matmul(
                    psf[:OUT, 0:NB],
                    WT_out[:, kc * OUT : (kc + 1) * OUT],
                    h1[kc],
                    start=(kc == 0),
                    stop=(kc == NKC - 1),
                )
            fo = wkp.tile([128, NB], F32, tag="fo")
            nc.vector.tensor_scalar_add(
                fo[:OUT, :], psf[:OUT, 0:NB], bot[:, 0:1]
            )
            nc.gpsimd.dma_start(
                out=out[:, :].rearrange("b o -> o b"), in_=fo[:OUT, :]
            )

    nc.compile()
    return nc


def kernel(**inputs):
    if "nc" not in _CACHE:
        _CACHE["nc"] = _build()
    nc = _CACHE["nc"]

    xb = np.asarray(inputs["xb"], dtype=np.float32)
    shared = {
        k: np.ascontiguousarray(np.asarray(inputs[k], dtype=np.float32))
        for k in (
            "W_ih0",
            "W_hh0",
            "b0",
            "W_ih1",
            "W_hh1",
            "b1",
            "W_out",
            "b_out",
        )
    }
    in_maps = []
    for i in range(NCORES):
        m = dict(shared)
        m["xb"] = np.ascontiguousarray(xb[i * BL : (i + 1) * BL])
        in_maps.append(m)

    trace = False
    try:
        trace = bool(int(os.environ.get("KERNEL_TRACE", "0")))
    except ValueError:
        pass
    try:
        res = run_bass_kernel_spmd(nc, in_maps, list(range(NCORES)), trace=trace)
    except ModuleNotFoundError:
        # no NTFF profiling hook in this container; fall back untraced
        res = run_bass_kernel_spmd(nc, in_maps, list(range(NCORES)))
    if trace:
        _CACHE["exec_time_ns"] = res.exec_time_ns
    return np.concatenate(
        [res.results[i]["out"] for i in range(NCORES)], axis=0
    )


# revision 16
# speedup vs baseline: 1.0343x; 1.0343x over previous
"""Trainium2 Bass kernel for a 2-layer LSTM (MnistModel frames).

Model: xb [2048, 8192] -> frames [T=64, B, 128] -> LSTM(128->512) ->
LSTM(512->512) -> last hidden -> Linear(512->10).

Sharding: data-parallel over batch (2048 -> 256 per core, 8 cores),
weights replicated.  Everything on-chip lives transposed ([feature,
batch]); weights/x are PE-transposed at startup and cast to bf16 (full
PE rate, fast-weight-load active).  All 64 transposed x timesteps stay
resident in SBUF as bf16 (32KB/partition).  Startup is interleaved with
the recurrence: weight staging DMAs are issued up front (by k-chunk),
transposes and x prep stream between the first timesteps, and the first
timestep of each layer skips the h-matmuls (h0 == 0).  Layer 1 runs LAG
timesteps behind layer 0 and the two layers' PSUM group pipelines are
emitted interleaved.
"""

import os
import sys

import numpy as np

for _p in ("/opt/trn_rl_repo", "/root/.axon_site/_ro/trn_rl_repo"):
    if os.path.isdir(_p) and _p not in sys.path:
        sys.path.insert(0, _p)

import concourse.bass as bass  # noqa: E402
import concourse.mybir as mybir  # noqa: E402
import concourse.tile as tile  # noqa: E402
from concourse import bacc  # noqa: E402
from concourse.bass_utils import run_bass_kernel_spmd  # noqa: E402
from concourse.masks import make_identity  # noqa: E402

F32 = mybir.dt.float32
BF16 = mybir.dt.bfloat16
AF = mybir.ActivationFunctionType

B, L, IN, H, OUT = 2048, 8192, 128, 512, 10
T = L // IN  # 64 timesteps
NCORES = 8
BL = B // NCORES  # 256 batch rows per core
G4 = 4 * H  # 2048 gate rows
NKC = H // 128  # 4 hidden k-chunks
NMC = G4 // 128  # 16 gate m-chunks
NB = BL  # matmul moving free dim
CHUNK_T = 8  # x timesteps per staging tile

_CACHE = {}


def _build(opts=None):
    _defaults = dict(
        wk_bufs=2,
        st_bufs=2,
        k_outer=True,
        lag=2,
    )
    _defaults.update(opts or {})
    opts = _defaults
    LAG = opts["lag"]
    assert LAG >= 1

    nc = bacc.Bacc()
    xb = nc.declare_dram_parameter("xb", [BL, L], F32, isOutput=False)
    W_ih0 = nc.declare_dram_parameter("W_ih0", [G4, IN], F32, isOutput=False)
    W_hh0 = nc.declare_dram_parameter("W_hh0", [G4, H], F32, isOutput=False)
    b0 = nc.declare_dram_parameter("b0", [G4], F32, isOutput=False)
    W_ih1 = nc.declare_dram_parameter("W_ih1", [G4, H], F32, isOutput=False)
    W_hh1 = nc.declare_dram_parameter("W_hh1", [G4, H], F32, isOutput=False)
    b1 = nc.declare_dram_parameter("b1", [G4], F32, isOutput=False)
    W_out = nc.declare_dram_parameter("W_out", [OUT, H], F32, isOutput=False)
    b_out = nc.declare_dram_parameter("b_out", [OUT], F32, isOutput=False)
    out = nc.declare_dram_parameter("out", [BL, OUT], F32, isOutput=True)

    with tile.TileContext(nc) as tc:
        with (
            tc.tile_pool(name="const", bufs=1) as const,
            tc.tile_pool(name="wstg", bufs=1) as wstg,
            tc.tile_pool(name="xstg", bufs=1) as xstg,
            tc.tile_pool(name="ps0", bufs=2, space="PSUM") as ps0,
            tc.tile_pool(name="ps1", bufs=2, space="PSUM") as ps1,
            tc.tile_pool(name="state", bufs=opts["st_bufs"]) as stp,
            tc.tile_pool(name="work", bufs=opts["wk_bufs"]) as wkp,
        ):
            ident = const.tile([128, 128], F32, tag="ident")
            make_identity(nc, ident)

            b0t = const.tile([128, NMC], F32, tag="b0t")
            nc.sync.dma_start(out=b0t, in_=b0[:].rearrange("(m p) -> p m", p=128))
            b1t = const.tile([128, NMC], F32, tag="b1t")
            nc.sync.dma_start(out=b1t, in_=b1[:].rearrange("(m p) -> p m", p=128))
            bot = const.tile([OUT, 1], F32, tag="bot")
            nc.sync.dma_start(out=bot, in_=b_out[:].rearrange("(p o) -> p o", o=1))

            # all transposed x timesteps live in SBUF as bf16
            xts = [
                const.tile([128, NB], BF16, tag=f"xts{t}", name=f"xts{t}")
                for t in range(T)
            ]

            # ---- weight staging: issue all DMAs up front (by k-chunk);
            # transposes stream later between recurrence timesteps.
            def load_start(wd, kdim, name):
                """Returns (bf16 lhsT tiles per k-chunk, staged fp32 tiles)."""
                nkc = kdim // 128
                wts = [
                    const.tile(
                        [128, G4], BF16, tag=f"{name}_{kc}", name=f"{name}_{kc}"
                    )
                    for kc in range(nkc)
                ]
                wv = wd[:, :].rearrange("(g p) k -> p g k", p=128)
                sts = []
                for kc in range(nkc):
                    st = wstg.tile(
                        [128, NMC, 128],
                        F32,
                        tag="wst",
                        name=f"st_{name}{kc}",
                        bufs=6,
                    )
                    nc.scalar.dma_start(
                        out=st, in_=wv[:, :, kc * 128 : (kc + 1) * 128]
                    )
                    sts.append(st)
                return wts, sts

            def wT_gen(wts, sts):
                """PE-transpose staged [128, 16, 128] chunks into bf16 lhsT
                tiles; yields between 512-col groups."""
                for kc, st in enumerate(sts):
                    for mg in range(NMC // 4):
                        pt = ps1.tile(
                            [128, 4 * NB], F32, tag="g", name="wpt"
                        )[:, 0:512]
                        for j in range(4):
                            mc = mg * 4 + j
                            nc.tensor.transpose(
                                pt[:, j * 128 : (j + 1) * 128],
                                st[:, mc, :],
                                ident,
                            )
                        nc.vector.tensor_copy(
                            wts[kc][:, mg * 512 : (mg + 1) * 512], pt
                        )
                        yield

            def x_group_gen(g):
                """Stage CHUNK_T timesteps of x, transpose to bf16 SBUF."""
                tch = g * CHUNK_T
                tend = min(tch + CHUNK_T, T)
                n = (tend - tch) * IN
                xs0 = xstg.tile([128, n], F32, tag="xs0", name=f"xs0_{g}", bufs=2)
                xs1 = xstg.tile([128, n], F32, tag="xs1", name=f"xs1_{g}", bufs=2)
                nc.sync.dma_start(out=xs0, in_=xb[0:128, tch * IN : tend * IN])
                nc.sync.dma_start(out=xs1, in_=xb[128:256, tch * IN : tend * IN])
                for t in range(tch, tend):
                    off = (t - tch) * IN
                    pt = ps0.tile([128, 4 * NB], F32, tag="g", name="xpt")[
                        :, 0:NB
                    ]
                    nc.tensor.transpose(pt[:, 0:128], xs0[:, off : off + IN], ident)
                    nc.tensor.transpose(
                        pt[:, 128:256], xs1[:, off : off + IN], ident
                    )
                    nc.vector.tensor_copy(xts[t], pt)
                    if t % 2 == 1:
                        yield

            wts_ih0, sts_ih0 = load_start(W_ih0, IN, "wih0")
            wts_hh0, sts_hh0 = load_start(W_hh0, H, "whh0")
            wts_ih1, sts_ih1 = load_start(W_ih1, H, "wih1")
            wts_hh1, sts_hh1 = load_start(W_hh1, H, "whh1")
            WT_ih0 = wts_ih0[0]

            def drive(gens):
                alive = list(gens)
                while alive:
                    for g in list(alive):
                        try:
                            next(g)
                        except StopIteration:
                            alive.remove(g)

            h0, c0, h1, c1 = [None] * NKC, [None] * NKC, [None] * NKC, [None] * NKC

            def lstm_step_gen(lname, pspool, pairs, c_prev, bt, res):
                """One LSTM layer timestep, transposed layout; yields after
                each of the NKC gate groups so two layers interleave.

                PSUM group tile holds (i|f|o|g) for one 128-slice of the
                hidden dim; all `pairs` (wT, rhs) accumulate into it, k-outer
                so late-arriving rhs chunks are needed late.  c_prev is None
                on the first timestep (h==0): f/v are skipped."""
                first = c_prev is None
                h_new, c_new = [], []
                n = len(pairs)
                for p in range(NKC):
                    ps = pspool.tile([128, 4 * NB], F32, tag="g", name=f"ps{lname}")
                    if opts["k_outer"]:
                        # k-outer across the two PSUM banks of the group
                        # tile: regions (i, o) accumulate k-interleaved,
                        # then (f, g).  Within a bank each region's
                        # accumulation stays contiguous.
                        for sub in range(2):
                            for idx, (wt, rhs) in enumerate(pairs):
                                for pos in (sub, sub + 2):
                                    gate = (0, 1, 3, 2)[pos]
                                    if first and gate == 1:
                                        continue  # f unused when c==0
                                    mc = gate * NKC + p
                                    nc.tensor.matmul(
                                        ps[:, pos * NB : (pos + 1) * NB],
                                        wt[:, mc * 128 : (mc + 1) * 128],
                                        rhs,
                                        start=(idx == 0),
                                        stop=(idx == n - 1),
                                        skip_group_check=True,
                                    )
                    else:
                        for pos, gate in enumerate((0, 1, 3, 2)):
                            if first and gate == 1:
                                continue
                            mc = gate * NKC + p
                            for idx, (wt, rhs) in enumerate(pairs):
                                nc.tensor.matmul(
                                    ps[:, pos * NB : (pos + 1) * NB],
                                    wt[:, mc * 128 : (mc + 1) * 128],
                                    rhs,
                                    start=(idx == 0),
                                    stop=(idx == n - 1),
                                )
                    # ACT order: i, f, g, o, then tanh(c) — so sigmoid(o)
                    # overlaps the DVE u/v/c chain and tanh(c) is reached
                    # as early as possible (h critical path).
                    sg = wkp.tile([128, 3 * NB], BF16, tag=f"sg{lname}")
                    nc.scalar.activation(
                        sg[:, 0:NB], ps[:, 0:NB], AF.Sigmoid,
                        bias=bt[:, p : p + 1],
                    )
                    if not first:
                        mcf = NKC + p
                        nc.scalar.activation(
                            sg[:, NB : 2 * NB], ps[:, NB : 2 * NB], AF.Sigmoid,
                            bias=bt[:, mcf : mcf + 1],
                        )
                    tg = wkp.tile([128, NB], BF16, tag=f"tg{lname}")
                    mcg = 2 * NKC + p
                    nc.scalar.activation(
                        tg, ps[:, 3 * NB : 4 * NB], AF.Tanh,
                        bias=bt[:, mcg : mcg + 1],
                    )
                    cn = stp.tile([128, NB], BF16, tag=f"c{lname}_{p}", bufs=2)
                    if first:
                        nc.vector.tensor_mul(cn, sg[:, 0:NB], tg)
                    else:
                        u = wkp.tile([128, NB], BF16, tag=f"u{lname}")
                        nc.vector.tensor_mul(u, sg[:, 0:NB], tg)
                        v = wkp.tile([128, NB], BF16, tag=f"v{lname}")
                        nc.vector.tensor_mul(v, sg[:, NB : 2 * NB], c_prev[p])
                        nc.vector.tensor_add(cn, u, v)
                    mco = 3 * NKC + p
                    nc.scalar.activation(
                        sg[:, 2 * NB : 3 * NB], ps[:, 2 * NB : 3 * NB],
                        AF.Sigmoid, bias=bt[:, mco : mco + 1],
                    )
                    th = wkp.tile([128, NB], BF16, tag=f"th{lname}")
                    nc.scalar.activation(th, cn, AF.Tanh)
                    hn = stp.tile(
                        [128, NB], BF16, tag=f"h{lname}_{p}",
                        bufs=(LAG + 2) if lname == "0" else 2,
                    )
                    nc.vector.tensor_mul(hn, sg[:, 2 * NB : 3 * NB], th)
                    h_new.append(hn)
                    c_new.append(cn)
                    yield
                res[lname] = (h_new, c_new)

            hs0 = {}  # t -> h0 chunks (consumed by layer 1 at t)

            def emit_l0(t):
                nonlocal h0, c0
                if t == 0:
                    pairs = [(WT_ih0, xts[0])]
                    cp = None
                else:
                    pairs = [(WT_ih0, xts[t])] + [
                        (wts_hh0[kc], h0[kc]) for kc in range(NKC)
                    ]
                    cp = c0
                res = {}
                yield from lstm_step_gen("0", ps0, pairs, cp, b0t, res)
                h0, c0 = res["0"]
                hs0[t] = h0

            def emit_l1(t):
                nonlocal h1, c1
                h0t = hs0.pop(t)
                if t == 0:
                    pairs = [(wts_ih1[kc], h0t[kc]) for kc in range(NKC)]
                    cp = None
                else:
                    pairs = [(wts_ih1[kc], h0t[kc]) for kc in range(NKC)] + [
                        (wts_hh1[kc], h1[kc]) for kc in range(NKC)
                    ]
                    cp = c1
                res = {}
                yield from lstm_step_gen("1", ps1, pairs, cp, b1t, res)
                h1, c1 = res["1"]

            # ---- prologue: x group 0 + W_ih0 transposed before t0; the
            # rest of the weight transposes and x groups stream between
            # the first timesteps (staging DMAs are already in flight).
            drive([x_group_gen(0)])
            drive([wT_gen(wts_ih0, sts_ih0)])
            drive([x_group_gen(1)])
            drive([emit_l0(0)])
            drive([wT_gen(wts_hh0, sts_hh0)])
            # NOTE: a wT_gen must COMPLETE in a drive() strictly before the
            # first emit_l1 that consumes its weights (instructions emitted
            # earlier in program order cannot depend on later writes).
            # ih1 finishes at t=1 (first l1 drive is t=LAG>=2); hh1 finishes
            # at t=2 and l1(0) in the same drive doesn't read hh1 (h1==0).
            sched = {
                1: [x_group_gen(2), wT_gen(wts_ih1, sts_ih1)],
                2: [x_group_gen(3), wT_gen(wts_hh1, sts_hh1)],
                3: [x_group_gen(4)],
                4: [x_group_gen(5)],
                5: [x_group_gen(6)],
                6: [x_group_gen(7)],
            }
            for t in range(1, LAG):
                drive([emit_l0(t)] + sched.pop(t, []))
            for t in range(LAG, T):
                drive([emit_l0(t), emit_l1(t - LAG)] + sched.pop(t, []))
            for t in range(T - LAG, T):
                drive([emit_l1(t)])

            # head: out.T [10, 256] = W_out @ h1T + b_out
            WT_out = const.tile([128, NKC * OUT], BF16, tag="wout")
            stw = wstg.tile([OUT, H], F32, tag="stw", name="st_wo")
            nc.scalar.dma_start(out=stw, in_=W_out[:, :])
            for kc in range(NKC):
                pt = ps0.tile([128, 4 * NB], F32, tag="g", name="ps0w")[
                    :, 0:OUT
                ]
                nc.tensor.transpose(
                    pt, stw[:, kc * 128 : (kc + 1) * 128], ident[:OUT, :OUT]
                )
                nc.vector.tensor_copy(WT_out[:, kc * OUT : (kc + 1) * OUT], pt)
            psf = ps0.tile([128, 4 * NB], F32, tag="g", name="psf")
            for kc in range(NKC):
                nc.tensor.matmul(
                    psf[:OUT, 0:NB],
                    WT_out[:, kc * OUT : (kc + 1) * OUT],
                    h1[kc],
                    start=(kc == 0),
                    stop=(kc == NKC - 1),
                )
            fo = wkp.tile([128, NB], F32, tag="fo")
            nc.vector.tensor_scalar_add(
                fo[:OUT, :], psf[:OUT, 0:NB], bot[:, 0:1]
            )
            nc.gpsimd.dma_start(
                out=out[:, :].rearrange("b o -> o b"), in_=fo[:OUT, :]
            )

    nc.compile()
    return nc


def kernel(**inputs):
    if "nc" not in _CACHE:
        _CACHE["nc"] = _build()
    nc = _CACHE["nc"]

    xb = np.asarray(inputs["xb"], dtype=np.float32)
    shared = {
        k: np.ascontiguousarray(np.asarray(inputs[k], dtype=np.float32))
        for k in (
            "W_ih0",
            "W_hh0",
            "b0",
            "W_ih1",
            "W_hh1",
            "b1",
            "W_out",
            "b_out",
        )
    }
    in_maps = []
    for i in range(NCORES):
        m = dict(shared)
        m["xb"] = np.ascontiguousarray(xb[i * BL : (i + 1) * BL])
        in_maps.append(m)

    trace = False
    try:
        trace = bool(int(os.environ.get("KERNEL_TRACE", "0")))
    except ValueError:
        pass
    try:
        res = run_bass_kernel_spmd(nc, in_maps, list(range(NCORES)), trace=trace)
    except ModuleNotFoundError:
        # no NTFF profiling hook in this container; fall back untraced
        res = run_bass_kernel_spmd(nc, in_maps, list(range(NCORES)))
    if trace:
        _CACHE["exec_time_ns"] = res.exec_time_ns
    return np.concatenate(
        [res.results[i]["out"] for i in range(NCORES)], axis=0
    )
